# revision 30
# baseline (speedup 1.0000x reference)
"""AttentiveFP GNN forward pass on 8 Trainium2 NeuronCores (Bass/Tile).

Strategy
--------
Molecules are sharded contiguously across 8 cores (batch is sorted).  Each
core's atoms are laid out in a padded node space where each 256-molecule
block starts at a fixed offset (identical schedule on every core, as required
for a shared SPMD NEFF).  Edges are owned by the core that owns their dst
atom and sorted by dst.  Per-edge work runs in 128-slot sub-tiles grouped by
256-node superwindows; segment softmax + weighted aggregation use
indicator-matrix matmuls accumulating in PSUM, normalized per node
(h[n] = sum_e exp(a_e) m_e / sum_e exp(a_e)).  Layer-1 source features are
host-pre-permuted into slot order (pure input data movement).  Between layer
1 and 2 the updated node features are AllGathered across cores; layer 2
fetches x2[src] rows with one [128,1]-indexed indirect DMA per sub-tile (the
only dynamic gather).  The molecule readout (2 timesteps) is fully local and
gather-free.
"""

import math
import os
import sys

sys.path.insert(0, "/opt/trn_rl_repo")

import numpy as np

import concourse.bass as bass
import concourse.mybir as mybir
import concourse.tile as tile
from concourse import bacc
from concourse.bass_utils import run_bass_kernel_spmd

F32 = mybir.dt.float32
I32 = mybir.dt.int32
AF = mybir.ActivationFunctionType
ALU = mybir.AluOpType

P = 128
SW = 256          # nodes per superwindow (2 psum halves)
MGN = 1024        # nodes per megagroup (GRU batch)
NCORES = 8
H = 128
NUM_TIMESTEPS = 2

FT = mybir.dt.bfloat16   # feature dtype for tables / matmul operands


class Cfg:
    pass


def _round_up(x, m):
    return (x + m - 1) // m * m


def _pack_slots(arr):
    """[S] -> [P, S//P] with slot s=j*P+p stored at [p, j]."""
    s = arr.shape[0]
    return np.ascontiguousarray(arr.reshape(s // P, P).T)


def preprocess(inputs, n_cores=NCORES):
    x = np.asarray(inputs["x"], np.float32)
    ea = np.asarray(inputs["edge_attr"], np.float32)
    ei = np.asarray(inputs["edge_index"], np.int32)
    batch = np.asarray(inputs["batch"], np.int32)
    n_atoms, in_dim = x.shape
    ed = ea.shape[1]
    n_mols = int(batch.max()) + 1

    b_core = max(1, n_mols // n_cores)
    mol_bounds = np.searchsorted(batch, np.arange(0, n_cores + 1) * b_core)
    mol_bounds[-1] = n_atoms
    a0 = mol_bounds[:-1].astype(np.int64)

    b_pad = _round_up(b_core + 1, SW)
    n_msw = b_pad // SW

    # block = 256 consecutive molecules; block row-start is FIXED across cores
    blk_cnt = np.zeros((n_cores, n_msw), np.int64)
    for c in range(n_cores):
        bl = batch[mol_bounds[c]:mol_bounds[c + 1]] - c * b_core
        blk_cnt[c] = np.bincount(bl // SW, minlength=n_msw)
    s_blk = _round_up(int(blk_cnt.max()), MGN)
    n_pad = n_msw * s_blk
    w_n = n_pad // P
    n_sw = n_pad // SW

    cfg = Cfg()
    cfg.n_atoms, cfg.in_dim, cfg.ed = n_atoms, in_dim, ed
    cfg.in_pad = in_dim + 1
    cfg.b_core, cfg.b_pad, cfg.n_msw, cfg.s_blk = b_core, b_pad, n_msw, s_blk
    cfg.n_pad, cfg.w_n, cfg.n_sw = n_pad, w_n, n_sw
    cfg.n_mg = n_pad // MGN
    cfg.mw_n = b_pad // P
    cfg.nch = 8
    cfg.ch_sub = 32
    rch = n_pad // cfg.nch

    # padded node position of each atom
    atom_owner = np.clip(
        np.searchsorted(mol_bounds, np.arange(n_atoms), side="right") - 1,
        0, n_cores - 1)
    bl_all = batch - atom_owner * b_core
    msw_all = bl_all // SW
    blk_start = np.zeros((n_cores, n_msw), np.int64)
    blk_start[:, 1:] = np.cumsum(blk_cnt, axis=1)[:, :-1]
    core_a0 = a0[atom_owner]
    within = np.arange(n_atoms) - core_a0 - blk_start[atom_owner, msw_all]
    node_pos = msw_all * s_blk + within      # padded position within core

    src, dst = ei[0].astype(np.int64), ei[1].astype(np.int64)
    e_owner = np.clip(np.searchsorted(mol_bounds, dst, side="right") - 1,
                      0, n_cores - 1)
    dst_pos = node_pos[dst]

    # agf row of a global atom (allgather chunk layout)
    kk = node_pos // rch
    agf_row = kk * (n_cores * rch) + atom_owner * rch + (node_pos - kk * rch)

    counts = np.zeros((n_cores, n_sw), np.int64)
    per_core = []
    for c in range(n_cores):
        sel = np.nonzero(e_owner == c)[0]
        dp = dst_pos[sel]
        order = np.argsort(dp, kind="stable")
        sel = sel[order]
        dp = dp[order]
        counts[c] = np.bincount(dp // SW, minlength=n_sw)
        per_core.append((sel, dp))
    esub_sw = np.maximum(1, np.ceil(counts.max(axis=0) / P).astype(np.int64))
    s_e = int(esub_sw.sum()) * P
    cfg.esub_sw = esub_sw
    cfg.s_e = s_e

    ftnp = np.dtype(mybir.dt.np(FT))
    xraw_pad = np.zeros((n_atoms, cfg.in_pad), np.float32)
    xraw_pad[:, :in_dim] = x

    in_maps = []
    slot_srcs, slot_valids = [], []
    for c in range(n_cores):
        sel, dp = per_core[c]
        slot_src = np.zeros(s_e, np.int64)
        slot_valid = np.zeros(s_e, bool)
        slot_dstrel = np.full(s_e, -1.0, np.float32)
        slot_ea = np.zeros((s_e, ed), np.float32)
        estart = np.concatenate([[0], np.cumsum(counts[c])]).astype(np.int64)
        base = 0
        for sw in range(n_sw):
            cnt = int(counts[c, sw])
            lo, hi = estart[sw], estart[sw] + cnt
            slot_src[base:base + cnt] = src[sel[lo:hi]]
            slot_valid[base:base + cnt] = True
            slot_dstrel[base:base + cnt] = dp[lo:hi] - sw * SW
            slot_ea[base:base + cnt] = ea[sel[lo:hi]]
            base += int(esub_sw[sw]) * P
        assert base == s_e
        slot_srcs.append(slot_src)
        slot_valids.append(slot_valid)

        xrt = np.zeros((cfg.in_pad, n_pad), np.float32)
        amask = atom_owner == c
        xrt[:in_dim, node_pos[amask]] = x[amask].T

        molrel = np.full(n_pad, -1.0, np.float32)
        molrel[node_pos[amask]] = (bl_all[amask] - msw_all[amask] * SW)

        in_maps.append({
            "xgT": np.ascontiguousarray(xraw_pad[slot_src].T).astype(ftnp),
            "xrawT_own": xrt.astype(ftnp),
            "eaT": np.ascontiguousarray(slot_ea.T).astype(ftnp),
            "dstrel": _pack_slots(slot_dstrel).astype(ftnp),
            "dstrel_row": slot_dstrel.reshape(1, -1).astype(ftnp),
            "molrel": _pack_slots(molrel).astype(ftnp),
            "molrel_row": molrel.reshape(1, -1).astype(ftnp),
        })

    # ---- all-to-all exchange plan for layer-2 source rows ----
    # Each receiver slot needs row x2aug[src]. Rows are exchanged in
    # per-(receiver, sender) chunks of CH rows; within a chunk rows are
    # unique source nodes sorted ascending (sender-local order), so the
    # sender gathers a sorted, deduplicated index list and the receiver
    # maps each slot to (sender_chunk, unique_rank).
    src_owner_all = atom_owner[np.concatenate(slot_srcs)].reshape(n_cores, s_e)
    CH = 0
    uniq_per_pair = []
    for c in range(n_cores):
        row = []
        for o in range(n_cores):
            m = (src_owner_all[c] == o) & slot_valids[c]
            nu = len(np.unique(node_pos[slot_srcs[c][m]]))
            row.append(nu)
            CH = max(CH, nu)
        uniq_per_pair.append(row)
    CH = _round_up(max(CH, 1), P)
    cfg.CH = CH
    send_idx = [np.zeros(n_cores * CH, np.int64) for _ in range(n_cores)]
    use_exch = os.environ.get("K_EXCH", "0") == "1"
    for c in range(n_cores):
        rrow = np.zeros(s_e, np.int64)
        for o in range(n_cores):
            m = np.nonzero((src_owner_all[c] == o) & slot_valids[c])[0]
            uniq, inv = np.unique(node_pos[slot_srcs[c][m]],
                                  return_inverse=True)
            rrow[m] = o * CH + inv
            send_idx[o][c * CH:c * CH + len(uniq)] = uniq
        if use_exch:
            in_maps[c]["gidx2"] = _pack_slots(rrow.astype(np.int32))
        else:
            in_maps[c]["gidx2"] = _pack_slots(
                agf_row[slot_srcs[c]].astype(np.int32))
    for o in range(n_cores):
        in_maps[o]["sgidx"] = _pack_slots(send_idx[o].astype(np.int32))

    # ---- weights / consts ----
    g = lambda q: np.asarray(inputs[q], np.float32)
    wm = {}
    wlin1t = np.zeros((cfg.in_pad, H), np.float32)
    wlin1t[:in_dim] = g("W_lin1").T
    wm["Wlin1T"] = wlin1t.astype(ftnp)
    wm["W1aT"] = np.ascontiguousarray(g("gate_W1")[:, :H].T).astype(ftnp)
    wm["W1bT"] = np.ascontiguousarray(g("gate_W1")[:, H:H + ed].T).astype(ftnp)
    wm["W2T"] = np.ascontiguousarray(g("gate_W2").T).astype(ftnp)
    wm["Wih1T"] = np.ascontiguousarray(g("gru1_Wih").T).astype(ftnp)
    wm["Whh1T"] = np.ascontiguousarray(g("gru1_Whh").T).astype(ftnp)
    wm["convWT"] = np.ascontiguousarray(g("conv_W").T).astype(ftnp)
    wm["Wih2T"] = np.ascontiguousarray(g("gru2_Wih").T).astype(ftnp)
    wm["Whh2T"] = np.ascontiguousarray(g("gru2_Whh").T).astype(ftnp)
    wm["molWT"] = np.ascontiguousarray(g("mol_W").T).astype(ftnp)
    wm["WihmT"] = np.ascontiguousarray(g("grum_Wih").T).astype(ftnp)
    wm["WhhmT"] = np.ascontiguousarray(g("grum_Whh").T).astype(ftnp)
    wm["Wlin2T"] = np.ascontiguousarray(g("W_lin2").T).astype(ftnp)
    wm["WheadT"] = np.ascontiguousarray(g("W_head").T).astype(ftnp)

    cols = {}

    def col(name, v):
        cols[name] = np.asarray(v, np.float32).reshape(H)

    col("b1", g("b_lin1"))
    col("attl", g("gate_att_l"))
    col("attr", g("gate_att_r"))
    col("gbias", g("gate_bias"))
    col("cattsrc", g("conv_W").T @ g("conv_att_src"))
    col("cattdst", g("conv_W").T @ g("conv_att_dst"))
    col("cbias", g("conv_bias"))
    col("cattmsrc", g("mol_W").T @ g("mol_att_src"))
    col("cattmdst", g("mol_W").T @ g("mol_att_dst"))
    col("molbias", g("mol_bias"))
    col("b2", g("b_lin2"))
    col("iop_lo", np.arange(P, dtype=np.float32))
    col("iop_hi", np.arange(P, dtype=np.float32) + P)
    for tag, pre in (("1", "gru1"), ("2", "gru2"), ("m", "grum")):
        bih = g(pre + "_bih")
        bhh = g(pre + "_bhh")
        col("brz_r" + tag, bih[:H] + bhh[:H])
        col("brz_z" + tag, bih[H:2 * H] + bhh[H:2 * H])
        # halved variants: sigmoid(y) computed as 0.5*tanh(0.5*y) + 0.5, so
        # the activation gets scale=0.5 and bias=0.5*(bih+bhh)
        col("brz_rh" + tag, 0.5 * (bih[:H] + bhh[:H]))
        col("brz_zh" + tag, 0.5 * (bih[H:2 * H] + bhh[H:2 * H]))
        col("bihn" + tag, bih[2 * H:])
        col("bhhn" + tag, bhh[2 * H:])
    order = sorted(cols)
    wm["cvec"] = np.stack([cols[q] for q in order], axis=1)
    cvec_idx = {q: i for i, q in enumerate(order)}
    # bf16 copies of the columns used as matmul operands / DVE compare inputs
    hcols = ["attl", "attr", "cattsrc", "cattdst", "cattmsrc", "cattmdst",
             "iop_lo", "iop_hi"]
    wm["cvech"] = np.stack([cols[q] for q in hcols], axis=1).astype(ftnp)
    cvech_idx = {q: i for i, q in enumerate(hcols)}

    iota = np.arange(P, dtype=np.float32)
    wm["iota_lo"] = np.tile(iota[None, :], (P, 1)).astype(ftnp)
    wm["iota_hi"] = (np.tile(iota[None, :], (P, 1)) + P).astype(ftnp)
    wm["identf32"] = np.eye(P, dtype=np.float32)
    wm["identity"] = np.eye(P, dtype=ftnp)

    for m in in_maps:
        m.update(wm)

    meta = {"cvec_idx": cvec_idx, "cvech_idx": cvech_idx,
            "b_head": float(np.asarray(inputs["b_head"]).reshape(-1)[0])}
    return cfg, in_maps, meta


# ---------------------------------------------------------------------------

class Builder:
    def __init__(self, cfg, cvec_idx, b_head, cvech_idx=None):
        self.cfg = cfg
        self.cvec_idx = cvec_idx
        self.cvech_idx = cvech_idx or {}
        self.b_head = b_head
        self.onecore = os.environ.get("K_ONECORE", "0") == "1"
        self.nc = bacc.Bacc("TRN2", target_bir_lowering=False, debug=False,
                            num_devices=1 if self.onecore else NCORES)

    def cc(self, name):
        i = self.cvec_idx[name]
        return self.scvec[:, i:i + 1]

    def cch(self, name):
        i = self.cvech_idx[name]
        return self.scvech[:, i:i + 1]

    def declare(self):
        nc, cfg = self.nc, self.cfg
        ei = lambda nm, sh, dt: nc.dram_tensor(nm, sh, dt, kind="ExternalInput")
        self.xgT = ei("xgT", [cfg.in_pad, cfg.s_e], FT)
        self.xrawT_own = ei("xrawT_own", [cfg.in_pad, cfg.n_pad], FT)
        self.eaT = ei("eaT", [cfg.ed, cfg.s_e], FT)
        self.gidx2 = ei("gidx2", [P, cfg.s_e // P], I32)
        self.sgidx = ei("sgidx", [P, NCORES * cfg.CH // P], I32)
        self.dstrel = ei("dstrel", [P, cfg.s_e // P], FT)
        self.dstrel_row = ei("dstrel_row", [1, cfg.s_e], FT)
        self.molrel = ei("molrel", [P, cfg.n_pad // P], FT)
        self.molrel_row = ei("molrel_row", [1, cfg.n_pad], FT)
        wn = {}
        for nm, sh in (("Wlin1T", [cfg.in_pad, H]), ("W1aT", [H, H]),
                       ("W1bT", [cfg.ed, H]), ("W2T", [H, H]),
                       ("Wih1T", [H, 3 * H]), ("Whh1T", [H, 3 * H]),
                       ("convWT", [H, H]), ("Wih2T", [H, 3 * H]),
                       ("Whh2T", [H, 3 * H]), ("molWT", [H, H]),
                       ("WihmT", [H, 3 * H]), ("WhhmT", [H, 3 * H]),
                       ("Wlin2T", [H, H]), ("WheadT", [H, 1]),
                       ("identity", [P, P])):
            wn[nm] = ei(nm, sh, FT)
        for nm, sh in (("iota_lo", [P, P]), ("iota_hi", [P, P]),
                       ("cvech", [P, len(self.cvech_idx)])):
            wn[nm] = ei(nm, sh, FT)
        for nm, sh in (("cvec", [P, len(self.cvec_idx)]),
                       ("identf32", [P, P])):
            wn[nm] = ei(nm, sh, F32)
        self.win = wn
        self.outp = nc.dram_tensor("out", [1, cfg.b_pad], F32,
                                   kind="ExternalOutput")
        self.x1T_d = nc.dram_tensor("x1T_d", [P, cfg.n_pad], FT)
        self.x2aug_d = nc.dram_tensor("x2aug_d", [cfg.n_pad, H + 1], FT)
        self.x2T_d = nc.dram_tensor("x2T_d", [P, cfg.n_pad], FT)
        self.use_exch = os.environ.get("K_EXCH", "0") == "1"
        if self.use_exch:
            self.send_d = nc.dram_tensor("send_d", [NCORES * cfg.CH, H + 1],
                                         FT)
            self.recv_d = nc.dram_tensor("recv_d", [NCORES * cfg.CH, H + 1],
                                         FT)
        else:
            self.recv_d = nc.dram_tensor("agf_d", [NCORES * cfg.n_pad, H + 1],
                                         FT, addr_space="Shared")

    def load_weights(self, tc, stack):
        nc = self.nc
        self.pw = stack.enter_context(tc.tile_pool(name="weights", bufs=1))
        self.pin = stack.enter_context(tc.tile_pool(name="pinned", bufs=1))

        def lc(nm):
            h = self.win[nm]
            t = self.pw.tile(list(h.shape), h.dtype, tag=nm, name=nm)
            nc.sync.dma_start(out=t[:, :], in_=h[:, :])
            return t

        self.sW = {nm: lc(nm) for nm in self.win}
        self.scvec = self.sW["cvec"]
        self.scvech = self.sW["cvech"]
        self.r1sb = self.pin.tile([P, self.cfg.w_n], FT, tag="r1sb",
                                  name="r1sb")
        self.a2sb = self.pin.tile([P, self.cfg.w_n], FT, tag="a2sb",
                                  name="a2sb")
        self.x3sb = self.pin.tile([P, self.cfg.w_n * (H + 1)], FT, tag="x3sb",
                                  name="x3sb")
        self.mrlsb = self.pin.tile([P, self.cfg.w_n], FT, tag="mrlsb",
                                   name="mrlsb")
        self.nc.sync.dma_start(out=self.mrlsb[:, :], in_=self.molrel[:, :])

    # ---------------- phase 0: lin1 + r1 on own atoms ----------------
    def phase0(self, tc):
        nc, cfg = self.nc, self.cfg
        WPM = MGN // P
        with tc.tile_pool(name="p0", bufs=2) as po, \
             tc.tile_pool(name="p0ps", bufs=2, space="PSUM") as pps:
            for mg in range(cfg.n_mg):
                m0 = mg * MGN
                xrt = po.tile([cfg.in_pad, MGN], FT, tag="xrt", name="xrt")
                nc.sync.dma_start(out=xrt[:, :],
                                  in_=self.xrawT_own[:, m0:m0 + MGN])
                x1mg = po.tile([P, MGN], FT, tag="x1mg", name="x1mg")
                for w8 in range(WPM):
                    ps = pps.tile([P, P], F32, tag="p0ps", name="p0ps",
                                  space="PSUM")
                    nc.tensor.matmul(ps[:, :], lhsT=self.sW["Wlin1T"][:, :],
                                     rhs=xrt[:, w8 * P:(w8 + 1) * P],
                                     start=True, stop=True)
                    nc.scalar.activation(x1mg[:, w8 * P:(w8 + 1) * P], ps[:, :],
                                         AF.Prelu, bias=self.cc("b1"),
                                         alpha=0.01)
                    psr = pps.tile([P, P], F32, tag="p0ps", name="psr",
                                   space="PSUM")
                    nc.tensor.matmul(psr[:, 0:1],
                                     lhsT=x1mg[:, w8 * P:(w8 + 1) * P],
                                     rhs=self.cch("attr"), start=True,
                                     stop=True)
                    nc.vector.tensor_copy(
                        self.r1sb[:, mg * WPM + w8:mg * WPM + w8 + 1],
                        psr[:, 0:1])
                nc.sync.dma_start(out=self.x1T_d[:, m0:m0 + MGN], in_=x1mg[:, :])

    # ---------------- GRU (feature-major, psum-accumulated) ----------------
    def gru(self, pool_sb, pool_ps, WihT, WhhT, tg, hT_ap, xprevT_ap, outT_ap,
            width):
        nc = self.nc
        nq = math.ceil(width / 512)
        for q in range(nq):
            sl = slice(q * 512, min((q + 1) * 512, width))
            qn = sl.stop - sl.start
            prz = pool_ps.tile([P, 512], F32, tag="gps", name="prz",
                               space="PSUM")
            nc.tensor.matmul(prz[:, :qn], lhsT=WihT[:, 0:H], rhs=hT_ap[:, sl],
                             start=True, stop=False)
            nc.tensor.matmul(prz[:, :qn], lhsT=WhhT[:, 0:H],
                             rhs=xprevT_ap[:, sl], start=False, stop=True)
            r = pool_sb.tile([P, 512], FT, tag="g_r", name="g_r")
            rt = pool_sb.tile([P, 512], FT, tag="g_rt", name="g_rt")
            nc.scalar.activation(rt[:, :qn], prz[:, :qn], AF.Tanh,
                                 scale=0.5, bias=self.cc("brz_rh" + tg))
            nc.vector.tensor_scalar(out=r[:, :qn], in0=rt[:, :qn],
                                    scalar1=0.5, scalar2=0.5,
                                    op0=ALU.mult, op1=ALU.add)
            pz = pool_ps.tile([P, 512], F32, tag="gps", name="pz", space="PSUM")
            nc.tensor.matmul(pz[:, :qn], lhsT=WihT[:, H:2 * H], rhs=hT_ap[:, sl],
                             start=True, stop=False)
            nc.tensor.matmul(pz[:, :qn], lhsT=WhhT[:, H:2 * H],
                             rhs=xprevT_ap[:, sl], start=False, stop=True)
            z = pool_sb.tile([P, 512], FT, tag="g_z", name="g_z")
            zt = pool_sb.tile([P, 512], FT, tag="g_rt", name="g_zt")
            nc.scalar.activation(zt[:, :qn], pz[:, :qn], AF.Tanh,
                                 scale=0.5, bias=self.cc("brz_zh" + tg))
            nc.vector.tensor_scalar(out=z[:, :qn], in0=zt[:, :qn],
                                    scalar1=0.5, scalar2=0.5,
                                    op0=ALU.mult, op1=ALU.add)
            pgn = pool_ps.tile([P, 512], F32, tag="gps", name="pgn",
                               space="PSUM")
            nc.tensor.matmul(pgn[:, :qn], lhsT=WihT[:, 2 * H:3 * H],
                             rhs=hT_ap[:, sl], start=True, stop=True)
            pgh = pool_ps.tile([P, 512], F32, tag="gps", name="pgh",
                               space="PSUM")
            nc.tensor.matmul(pgh[:, :qn], lhsT=WhhT[:, 2 * H:3 * H],
                             rhs=xprevT_ap[:, sl], start=True, stop=True)
            hn = pool_sb.tile([P, 512], FT, tag="g_t", name="hn")
            nc.vector.tensor_scalar(out=hn[:, :qn], in0=pgh[:, :qn],
                                    scalar1=self.cc("bhhn" + tg), scalar2=None,
                                    op0=ALU.add)
            rn = pool_sb.tile([P, 512], FT, tag="g_t", name="rn")
            nc.vector.tensor_tensor(out=rn[:, :qn], in0=r[:, :qn],
                                    in1=hn[:, :qn], op=ALU.mult)
            pre_n = pool_sb.tile([P, 512], FT, tag="g_t", name="pre_n")
            nc.vector.tensor_tensor(out=pre_n[:, :qn], in0=pgn[:, :qn],
                                    in1=rn[:, :qn], op=ALU.add)
            n_ = pool_sb.tile([P, 512], FT, tag="g_n", name="g_n")
            nc.scalar.activation(n_[:, :qn], pre_n[:, :qn], AF.Tanh,
                                 bias=self.cc("bihn" + tg))
            d = pool_sb.tile([P, 512], FT, tag="g_t", name="d")
            nc.vector.tensor_tensor(out=d[:, :qn], in0=xprevT_ap[:, sl],
                                    in1=n_[:, :qn], op=ALU.subtract)
            zd = pool_sb.tile([P, 512], FT, tag="g_t", name="zd")
            nc.vector.tensor_tensor(out=zd[:, :qn], in0=z[:, :qn],
                                    in1=d[:, :qn], op=ALU.mult)
            xs = pool_sb.tile([P, 512], FT, tag="g_t", name="xs")
            nc.vector.tensor_tensor(out=xs[:, :qn], in0=n_[:, :qn],
                                    in1=zd[:, :qn], op=ALU.add)
            nc.vector.tensor_scalar(out=outT_ap[:, sl], in0=xs[:, :qn],
                                    scalar1=0.0, scalar2=None, op0=ALU.max)

    def elu(self, pool_sb, ps_ap, bias_col, out_ap, qn):
        nc = self.nc
        hb = pool_sb.tile([P, 512], FT, tag="e_hb", name="e_hb")
        nc.vector.tensor_scalar(out=hb[:, :qn], in0=ps_ap, scalar1=bias_col,
                                scalar2=None, op0=ALU.add)
        el = pool_sb.tile([P, 512], FT, tag="e_t", name="e_el")
        nc.vector.tensor_scalar(out=el[:, :qn], in0=hb[:, :qn], scalar1=0.0,
                                scalar2=None, op0=ALU.min)
        ex = pool_sb.tile([P, 512], FT, tag="e_ex", name="e_ex")
        nc.scalar.activation(ex[:, :qn], el[:, :qn], AF.Exp)
        mx = pool_sb.tile([P, 512], FT, tag="e_t", name="e_mx")
        nc.vector.tensor_scalar(out=mx[:, :qn], in0=hb[:, :qn], scalar1=0.0,
                                scalar2=None, op0=ALU.max)
        sm = pool_sb.tile([P, 512], FT, tag="e_sm", name="e_sm")
        nc.vector.tensor_tensor(out=sm[:, :qn], in0=mx[:, :qn], in1=ex[:, :qn],
                                op=ALU.add)
        nc.vector.tensor_scalar(out=out_ap, in0=sm[:, :qn], scalar1=-1.0,
                                scalar2=None, op0=ALU.add)

    # ---------------- edge layer (1 or 2) ----------------
    def edge_layer(self, tc, layer):
        nc, cfg = self.nc, self.cfg
        WPM = MGN // P
        esub = cfg.esub_sw
        n_sub_total = cfg.s_e // P
        sub_sw = np.repeat(np.arange(cfg.n_sw), esub)
        sw_first = np.concatenate([[0], np.cumsum(esub)])
        ch_sub = cfg.ch_sub
        sidentf32 = self.sW["identf32"]
        MMX = nc.tensor.matmul

        with tc.tile_pool(name=f"l{layer}g", bufs=2) as pg, \
             tc.tile_pool(name=f"l{layer}rg", bufs=6) as prgather, \
             tc.tile_pool(name=f"l{layer}s", bufs=3) as psub, \
             tc.tile_pool(name=f"l{layer}mg", bufs=2) as pmg, \
             tc.tile_pool(name=f"l{layer}eps", bufs=4, space="PSUM") as pps, \
             tc.tile_pool(name=f"l{layer}hsw", bufs=1, space="PSUM") as phsw, \
             tc.tile_pool(name=f"l{layer}gps", bufs=2, space="PSUM") as ppsg:

            aggT_bufs = {}
            mg_done = {}
            hsw_tiles = {}

            def sw_epilogue(sw, tiles):
                mg = (sw * SW) // MGN
                if mg not in aggT_bufs:
                    aggT_bufs[mg] = pmg.tile([P, MGN], FT, tag="aggT",
                                             name="aggT")
                aggT = aggT_bufs[mg]
                for half, hps in enumerate(tiles):
                    w = 2 * sw + half
                    off = (w * P) % MGN
                    srec = psub.tile([P, 1], F32, tag="srec", name="srec")
                    nc.vector.tensor_scalar(out=srec[:, :], in0=hps[:, H:H + 1],
                                            scalar1=1e-16, scalar2=None,
                                            op0=ALU.add)
                    nc.vector.reciprocal(srec[:, :], srec[:, :])
                    aggN = psub.tile([P, H], FT, tag="aggN", name="aggN")
                    nc.vector.tensor_scalar(out=aggN[:, :], in0=hps[:, :H],
                                            scalar1=srec[:, :], scalar2=None,
                                            op0=ALU.mult)
                    pst = pps.tile([P, P], FT, tag="eps", name="aggps",
                                   space="PSUM")
                    nc.tensor.transpose(pst[:, :], aggN[:, :], self.sW["identity"][:, :])
                    nc.scalar.copy(aggT[:, off:off + P], pst[:, :])
                mg_done[mg] = mg_done.get(mg, 0) + 1
                if mg_done[mg] == MGN // SW:
                    mg_epilogue(mg, aggT_bufs.pop(mg))

            def mg_epilogue(mg, aggT):
                m0 = mg * MGN
                hT = pmg.tile([P, MGN], FT, tag="hT", name="hT")
                for q in range(MGN // 512):
                    ps = ppsg.tile([P, 512], F32, tag="gps", name="wps",
                                   space="PSUM")
                    MMX(ps[:, :], lhsT=(self.sW["W2T"] if layer == 1
                                        else self.sW["convWT"])[:, :],
                        rhs=aggT[:, q * 512:(q + 1) * 512], start=True,
                        stop=True)
                    if layer == 1:
                        self.elu(pmg, ps[:, :], self.cc("gbias"),
                                 hT[:, q * 512:(q + 1) * 512], 512)
                    else:
                        nc.scalar.activation(hT[:, q * 512:(q + 1) * 512],
                                             ps[:, :], AF.Relu,
                                             bias=self.cc("cbias"))
                xprevT = pmg.tile([P, MGN], FT, tag="xprevT", name="xprevT")
                nc.sync.dma_start(
                    out=xprevT[:, :],
                    in_=(self.x1T_d if layer == 1
                         else self.x2T_d)[:, m0:m0 + MGN])
                xnewT = pmg.tile([P, MGN], FT, tag="xnewT", name="xnewT")
                tg = "1" if layer == 1 else "2"
                self.gru(pmg, ppsg, self.sW["Wih" + tg + "T"],
                         self.sW["Whh" + tg + "T"], tg, hT[:, :], xprevT[:, :],
                         xnewT[:, :], MGN)
                if layer == 1:
                    aug = pmg.tile([P, WPM * (H + 1)], FT, tag="aug",
                                   name="aug")
                else:
                    aug = self.x3sb[:, mg * WPM * (H + 1):
                                    (mg + 1) * WPM * (H + 1)]
                for w8 in range(WPM):
                    sl = slice(w8 * P, (w8 + 1) * P)
                    pst = ppsg.tile([P, 512], FT, tag="gps", name="tps",
                                    space="PSUM")
                    nc.tensor.transpose(pst[:, :P], xnewT[:, sl],
                                        self.sW["identity"][:, :])
                    nc.scalar.copy(aug[:, w8 * (H + 1):w8 * (H + 1) + H],
                                   pst[:, :P])
                    psc = ppsg.tile([P, 512], F32, tag="gps", name="cps",
                                    space="PSUM")
                    MMX(psc[:, 0:1], lhsT=xnewT[:, sl],
                        rhs=self.cch("cattsrc") if layer == 1
                        else self.cch("cattmsrc"), start=True, stop=True)
                    nc.vector.tensor_copy(
                        aug[:, w8 * (H + 1) + H:w8 * (H + 1) + H + 1],
                        psc[:, 0:1])
                    if layer == 1:
                        psd = ppsg.tile([P, 512], F32, tag="gps", name="dps",
                                        space="PSUM")
                        MMX(psd[:, 0:1], lhsT=xnewT[:, sl],
                            rhs=self.cch("cattdst"), start=True, stop=True)
                        nc.vector.tensor_copy(
                            self.a2sb[:, mg * WPM + w8:mg * WPM + w8 + 1],
                            psd[:, 0:1])
                if layer == 1:
                    dview = self.x2aug_d[m0:m0 + MGN, :].rearrange(
                        "(w p) f -> p w f", p=P)
                    nc.sync.dma_start(
                        out=dview,
                        in_=aug[:, :].rearrange("p (w f) -> p w f", w=WPM))
                    nc.sync.dma_start(out=self.x2T_d[:, m0:m0 + MGN],
                                      in_=xnewT[:, :])

            for ch in range(math.ceil(n_sub_total / ch_sub)):
                st0 = ch * ch_sub
                st1 = min(st0 + ch_sub, n_sub_total)
                k = st1 - st0
                if layer == 1:
                    xgc = pg.tile([cfg.in_pad, ch_sub * P], FT, tag="xgc",
                                  name="xgc")
                    nc.sync.dma_start(out=xgc[:, :k * P],
                                      in_=self.xgT[:, st0 * P:st1 * P])
                    eac = pg.tile([cfg.ed, ch_sub * P], FT, tag="eac",
                                  name="eac")
                    nc.sync.dma_start(out=eac[:, :k * P],
                                      in_=self.eaT[:, st0 * P:st1 * P])
                drc = pg.tile([P, ch_sub], FT, tag="drc", name="drc")
                nc.sync.dma_start(out=drc[:, :k], in_=self.dstrel[:, st0:st1])
                if layer == 2:
                    gix = pg.tile([P, ch_sub], I32, tag="gix", name="gix")
                    nc.sync.dma_start(out=gix[:, :k],
                                      in_=self.gidx2[:, st0:st1])
                drcrep = pg.tile([P, ch_sub * P], FT, tag="drcrep",
                                 name="drcrep")
                nc.sync.dma_start(
                    out=drcrep[:, :k * P],
                    in_=self.dstrel_row[:, st0 * P:st1 * P].to_broadcast(
                        [P, k * P]))

                for st in range(st0, st1):
                    j = st - st0
                    sw = int(sub_sw[st])
                    first = st == sw_first[sw]
                    last = st == sw_first[sw + 1] - 1
                    if first:
                        hsw_tiles[sw] = (
                            phsw.tile([P, H + 1], F32, tag="hswlo",
                                      name="hswlo", space="PSUM"),
                            phsw.tile([P, H + 1], F32, tag="hswhi",
                                      name="hswhi", space="PSUM"))
                    hlo, hhi = hsw_tiles[sw]

                    mtlo = psub.tile([P, P], FT, tag="mtlo", name="mtlo")
                    nc.vector.tensor_tensor(
                        out=mtlo[:, :],
                        in0=self.cch("iop_lo").to_broadcast([P, P]),
                        in1=drcrep[:, j * P:(j + 1) * P], op=ALU.is_equal)
                    mthi = psub.tile([P, P], FT, tag="mthi", name="mthi")
                    nc.vector.tensor_tensor(
                        out=mthi[:, :],
                        in0=self.cch("iop_hi").to_broadcast([P, P]),
                        in1=drcrep[:, j * P:(j + 1) * P], op=ALU.is_equal)

                    concat = psub.tile([P, H + 1], FT, tag="concat",
                                       name="concat")
                    apsum = pps.tile([P, P], F32, tag="eps", name="apsum",
                                     space="PSUM")
                    if layer == 1:
                        psx = pps.tile([P, P], F32, tag="eps", name="psx",
                                       space="PSUM")
                        MMX(psx[:, :], lhsT=self.sW["Wlin1T"][:, :],
                            rhs=xgc[:, j * P:(j + 1) * P], start=True,
                            stop=True)
                        xj1T = psub.tile([P, P], FT, tag="xj1T", name="xj1T")
                        nc.scalar.activation(xj1T[:, :], psx[:, :], AF.Prelu,
                                             bias=self.cc("b1"), alpha=0.01)
                        psh = pps.tile([P, P], F32, tag="eps", name="psh",
                                       space="PSUM")
                        MMX(psh[:, :], lhsT=self.sW["W1aT"][:, :],
                            rhs=xj1T[:, :], start=True, stop=False)
                        MMX(psh[:, :], lhsT=self.sW["W1bT"][:, :],
                            rhs=eac[:, j * P:(j + 1) * P], start=False,
                            stop=True)
                        heT = psub.tile([P, P], FT, tag="heT", name="heT")
                        nc.scalar.activation(heT[:, :], psh[:, :], AF.Prelu,
                                             alpha=0.01)
                        MMX(apsum[:, 0:1], lhsT=heT[:, :], rhs=self.cch("attl"),
                            start=True, stop=False)
                        MMX(apsum[:, 0:1], lhsT=mtlo[:, :],
                            rhs=self.r1sb[:, 2 * sw:2 * sw + 1], start=False,
                            stop=False)
                        MMX(apsum[:, 0:1], lhsT=mthi[:, :],
                            rhs=self.r1sb[:, 2 * sw + 1:2 * sw + 2],
                            start=False, stop=True)
                        a1 = psub.tile([P, 1], F32, tag="a1", name="a1")
                        nc.scalar.activation(a1[:, :], apsum[:, 0:1], AF.Prelu,
                                             alpha=0.01)
                        e1 = psub.tile([P, 1], F32, tag="e1", name="e1")
                        nc.scalar.activation(e1[:, :], a1[:, :], AF.Exp)
                        nc.vector.tensor_copy(concat[:, H:H + 1], e1[:, :])
                        pst2 = pps.tile([P, P], FT, tag="eps", name="pst2",
                                        space="PSUM")
                        nc.tensor.transpose(pst2[:, :], xj1T[:, :],
                                            self.sW["identity"][:, :])
                        nc.vector.tensor_scalar(
                            out=concat[:, :H], in0=pst2[:, :],
                            scalar1=e1[:, :], scalar2=None,
                            op0=ALU.mult)
                    else:
                        rg = prgather.tile([P, H + 1], FT, tag="rg", name="rg")
                        nc.gpsimd.indirect_dma_start(
                            out=rg[:, :], out_offset=None,
                            in_=self.recv_d[:, :],
                            in_offset=bass.IndirectOffsetOnAxis(
                                ap=gix[:, j:j + 1], axis=0))
                        MMX(apsum[:, 0:1], lhsT=mtlo[:, :],
                            rhs=self.a2sb[:, 2 * sw:2 * sw + 1], start=True,
                            stop=False)
                        MMX(apsum[:, 0:1], lhsT=mthi[:, :],
                            rhs=self.a2sb[:, 2 * sw + 1:2 * sw + 2],
                            start=False, stop=True)
                        apre = psub.tile([P, 1], F32, tag="a1", name="apre")
                        nc.vector.tensor_tensor(out=apre[:, :],
                                                in0=apsum[:, 0:1],
                                                in1=rg[:, H:H + 1], op=ALU.add)
                        a1 = psub.tile([P, 1], F32, tag="a1b", name="a1b")
                        nc.scalar.activation(a1[:, :], apre[:, :], AF.Prelu,
                                             alpha=0.01)
                        e1 = psub.tile([P, 1], F32, tag="e1", name="e1b")
                        nc.scalar.activation(e1[:, :], a1[:, :], AF.Exp)
                        nc.vector.tensor_copy(concat[:, H:H + 1], e1[:, :])
                        nc.vector.tensor_scalar(
                            out=concat[:, :H], in0=rg[:, :H],
                            scalar1=e1[:, :], scalar2=None,
                            op0=ALU.mult)

                    mlo = psub.tile([P, P], FT, tag="mlo", name="mlo")
                    nc.vector.tensor_tensor(
                        out=mlo[:, :],
                        in0=drc[:, j:j + 1].to_broadcast([P, P]),
                        in1=self.sW["iota_lo"][:, :], op=ALU.is_equal)
                    mhi = psub.tile([P, P], FT, tag="mhi", name="mhi")
                    nc.vector.tensor_tensor(
                        out=mhi[:, :],
                        in0=drc[:, j:j + 1].to_broadcast([P, P]),
                        in1=self.sW["iota_hi"][:, :], op=ALU.is_equal)
                    MMX(hlo[:, :], lhsT=mlo[:, :], rhs=concat[:, :],
                        start=first, stop=last, skip_group_check=True)
                    MMX(hhi[:, :], lhsT=mhi[:, :], rhs=concat[:, :],
                        start=first, stop=last, skip_group_check=True)
                    if last:
                        sw_epilogue(sw, hsw_tiles.pop(sw))

    def exchange(self, tc):
        """Gather own x2aug rows into per-destination chunks, AllToAll them.

        send_d chunk c holds the unique source rows receiver c needs from
        this core, sorted by local row; recv_d chunk o (after AllToAll)
        holds the rows this core needs from sender o, indexed by gidx2."""
        nc, cfg = self.nc, self.cfg
        rows = NCORES * cfg.CH
        GC = 32                      # offset columns per indirect gather
        # (P * GC descriptors per op; dynamic-DMA scratch caps at 16384)
        rows_per = P * GC
        with tc.tile_pool(name="exg", bufs=2) as pex, \
             tc.tile_pool(name="exgi", bufs=1) as pexi:
            sg = pexi.tile([P, rows // P], I32, tag="sg", name="sg")
            nc.sync.dma_start(out=sg[:, :], in_=self.sgidx[:, :])
            for it in range(math.ceil(rows / rows_per)):
                j0 = it * GC
                j1 = min(j0 + GC, rows // P)
                k = j1 - j0
                rowst = pex.tile([P, GC * (H + 1)], FT, tag="exrows",
                                 name="exrows")
                nc.gpsimd.indirect_dma_start(
                    out=rowst[:, :k * (H + 1)].rearrange(
                        "p (j f) -> p j f", j=k),
                    out_offset=None,
                    in_=self.x2aug_d[:, :],
                    in_offset=bass.IndirectOffsetOnAxis(
                        ap=sg[:, j0:j1], axis=0))
                nc.sync.dma_start(
                    out=self.send_d[j0 * P:j1 * P, :].rearrange(
                        "(j p) f -> p j f", p=P),
                    in_=rowst[:, :k * (H + 1)].rearrange(
                        "p (j f) -> p j f", j=k))
        if self.onecore or os.environ.get("K_NOCC", "0") == "1":
            nc.sync.dma_start(out=self.recv_d[:, :], in_=self.send_d[:, :])
        else:
            nc.gpsimd.collective_compute(
                "AllToAll", ALU.bypass,
                replica_groups=[list(range(NCORES))],
                ins=[self.send_d[:, :].opt()],
                outs=[self.recv_d[:, :].opt()])

    def allgather(self):
        nc, cfg = self.nc, self.cfg
        rch = cfg.n_pad // cfg.nch
        if self.onecore:
            # analysis mode: stand-in DMA with the same footprint per chunk
            for q in range(cfg.nch):
                for r in range(NCORES):
                    nc.sync.dma_start(
                        out=self.recv_d[(q * NCORES + r) * rch:
                                        (q * NCORES + r + 1) * rch, :],
                        in_=self.x2aug_d[q * rch:(q + 1) * rch, :])
            return
        for q in range(cfg.nch):
            nc.gpsimd.collective_compute(
                "AllGather", ALU.bypass,
                replica_groups=[list(range(NCORES))],
                ins=[self.x2aug_d[q * rch:(q + 1) * rch, :].opt()],
                outs=[self.recv_d[q * NCORES * rch:(q + 1) * NCORES * rch,
                                  :].opt()])

    # ---------------- readout ----------------
    def readout(self, tc):
        nc, cfg = self.nc, self.cfg
        n_sub_total = cfg.n_pad // P
        sub_per_blk = cfg.s_blk // P
        ch_sub = cfg.ch_sub
        sidentf32 = self.sW["identf32"]
        MMX = nc.tensor.matmul

        with tc.tile_pool(name="ro", bufs=2) as pro, \
             tc.tile_pool(name="roS", bufs=3) as prs, \
             tc.tile_pool(name="roT", bufs=1) as proT, \
             tc.tile_pool(name="rog", bufs=3) as prg, \
             tc.tile_pool(name="rops", bufs=3, space="PSUM") as prps, \
             tc.tile_pool(name="rohm", bufs=1, space="PSUM") as phm:

            outT = proT.tile([P, cfg.b_pad], FT, tag="outT", name="outT")
            admol = [proT.tile([P, cfg.mw_n], FT, tag=f"admol{t}",
                               name=f"admol{t}")
                     for t in range(NUM_TIMESTEPS)]

            def mol_sw_epilogue(tstep, msw, tiles):
                for half, hps in enumerate(tiles):
                    mw = 2 * msw + half
                    off = mw * P
                    if tstep < 0:
                        agg = prg.tile([P, H], FT, tag="maggN", name="magg")
                        nc.vector.tensor_copy(agg[:, :], hps[:, :H])
                        pst = prps.tile([P, P], FT, tag="gps", name="mtps",
                                        space="PSUM")
                        nc.tensor.transpose(pst[:, :], agg[:, :],
                                            self.sW["identity"][:, :])
                        nc.scalar.activation(outT[:, off:off + P], pst[:, :],
                                             AF.Relu)
                        continue
                    srec = prg.tile([P, 1], F32, tag="msrec", name="msrec")
                    nc.vector.tensor_scalar(out=srec[:, :], in0=hps[:, H:H + 1],
                                            scalar1=1e-16, scalar2=None,
                                            op0=ALU.add)
                    nc.vector.reciprocal(srec[:, :], srec[:, :])
                    aggN = prg.tile([P, H], FT, tag="maggN", name="maggN")
                    nc.vector.tensor_scalar(out=aggN[:, :], in0=hps[:, :H],
                                            scalar1=srec[:, :], scalar2=None,
                                            op0=ALU.mult)
                    pst = prps.tile([P, P], FT, tag="gps", name="mtps2",
                                    space="PSUM")
                    nc.tensor.transpose(pst[:, :], aggN[:, :], self.sW["identity"][:, :])
                    aggT = prg.tile([P, P], FT, tag="maggT", name="maggT")
                    nc.scalar.copy(aggT[:, :], pst[:, :])
                    psh = prps.tile([P, P], F32, tag="gps", name="mhps",
                                    space="PSUM")
                    MMX(psh[:, :], lhsT=self.sW["molWT"][:, :], rhs=aggT[:, :],
                        start=True, stop=True)
                    hel = prg.tile([P, P], FT, tag="mhel", name="mhel")
                    self.elu(prg, psh[:, :], self.cc("molbias"), hel[:, :P], P)
                    self.gru(prg, prps, self.sW["WihmT"], self.sW["WhhmT"],
                             "m", hel[:, :], outT[:, off:off + P],
                             outT[:, off:off + P], P)

            def mol_pass(tstep):
                hm_tiles = {}
                for ch in range(math.ceil(n_sub_total / ch_sub)):
                    st0 = ch * ch_sub
                    st1 = min(st0 + ch_sub, n_sub_total)
                    k = st1 - st0
                    rg = self.x3sb[:, st0 * (H + 1):st1 * (H + 1)]
                    mrl = self.mrlsb[:, st0:st1]
                    if tstep >= 0:
                        mrlrep = pro.tile([P, ch_sub * P], FT, tag="mrlrep",
                                          name="mrlrep")
                        nc.sync.dma_start(
                            out=mrlrep[:, :k * P],
                            in_=self.molrel_row[:, st0 * P:st1 * P]
                            .to_broadcast([P, k * P]))
                    for st in range(st0, st1):
                        j = st - st0
                        msw = st // sub_per_blk
                        first = st % sub_per_blk == 0
                        last = (st + 1) % sub_per_blk == 0
                        if first:
                            hm_tiles[msw] = (
                                phm.tile([P, H + 1], F32, tag="hmlo",
                                         name="hmlo", space="PSUM"),
                                phm.tile([P, H + 1], F32, tag="hmhi",
                                         name="hmhi", space="PSUM"))
                        hlo, hhi = hm_tiles[msw]
                        if tstep >= 0:
                            mtlo = prs.tile([P, P], FT, tag="mmtlo",
                                            name="mmtlo")
                            nc.vector.tensor_tensor(
                                out=mtlo[:, :],
                                in0=self.cch("iop_lo").to_broadcast([P, P]),
                                in1=mrlrep[:, j * P:(j + 1) * P],
                                op=ALU.is_equal)
                            mthi = prs.tile([P, P], FT, tag="mmthi",
                                            name="mmthi")
                            nc.vector.tensor_tensor(
                                out=mthi[:, :],
                                in0=self.cch("iop_hi").to_broadcast([P, P]),
                                in1=mrlrep[:, j * P:(j + 1) * P],
                                op=ALU.is_equal)
                            apsum = prps.tile([P, P], F32, tag="gps",
                                              name="mapsum", space="PSUM")
                            MMX(apsum[:, 0:1], lhsT=mtlo[:, :],
                                rhs=admol[tstep][:, 2 * msw:2 * msw + 1],
                                start=True, stop=False)
                            MMX(apsum[:, 0:1], lhsT=mthi[:, :],
                                rhs=admol[tstep][:, 2 * msw + 1:2 * msw + 2],
                                start=False, stop=True)
                            apre = prs.tile([P, 1], F32, tag="ma1",
                                            name="mapre")
                            nc.vector.tensor_tensor(
                                out=apre[:, :], in0=apsum[:, 0:1],
                                in1=rg[:, j * (H + 1) + H:j * (H + 1) + H + 1],
                                op=ALU.add)
                            a1 = prs.tile([P, 1], F32, tag="ma1b", name="ma1b")
                            nc.scalar.activation(a1[:, :], apre[:, :], AF.Prelu,
                                                 alpha=0.01)
                            concat = prs.tile([P, H + 1], FT, tag="mconcat",
                                              name="mconcat")
                            e1 = prs.tile([P, 1], F32, tag="me1", name="me1")
                            nc.scalar.activation(e1[:, :], a1[:, :], AF.Exp)
                            nc.vector.tensor_copy(concat[:, H:H + 1], e1[:, :])
                            nc.vector.tensor_scalar(
                                out=concat[:, :H],
                                in0=rg[:, j * (H + 1):j * (H + 1) + H],
                                scalar1=e1[:, :], scalar2=None,
                                op0=ALU.mult)
                            rhs = concat[:, :]
                        else:
                            rhs = rg[:, j * (H + 1):(j + 1) * (H + 1)]
                        mlo = prs.tile([P, P], FT, tag="mmlo", name="mmlo")
                        nc.vector.tensor_tensor(
                            out=mlo[:, :],
                            in0=mrl[:, j:j + 1].to_broadcast([P, P]),
                            in1=self.sW["iota_lo"][:, :], op=ALU.is_equal)
                        mhi = prs.tile([P, P], FT, tag="mmhi", name="mmhi")
                        nc.vector.tensor_tensor(
                            out=mhi[:, :],
                            in0=mrl[:, j:j + 1].to_broadcast([P, P]),
                            in1=self.sW["iota_hi"][:, :], op=ALU.is_equal)
                        MMX(hlo[:, :], lhsT=mlo[:, :], rhs=rhs, start=first,
                            stop=last, skip_group_check=True)
                        MMX(hhi[:, :], lhsT=mhi[:, :], rhs=rhs, start=first,
                            stop=last, skip_group_check=True)
                        if last:
                            mol_sw_epilogue(tstep, msw, hm_tiles.pop(msw))

            def write_admol(tstep):
                for mw in range(cfg.mw_n):
                    ps = prps.tile([P, P], F32, tag="gps", name="amps",
                                   space="PSUM")
                    MMX(ps[:, 0:1], lhsT=outT[:, mw * P:(mw + 1) * P],
                        rhs=self.cch("cattmdst"), start=True, stop=True)
                    nc.vector.tensor_copy(admol[tstep][:, mw:mw + 1],
                                          ps[:, 0:1])

            import os
            ro_lvl = int(os.environ.get("K_RO", "3"))
            mol_pass(-1)
            if ro_lvl >= 2:
                for t in range(min(NUM_TIMESTEPS, ro_lvl - 1)):
                    write_admol(t)
                    mol_pass(t)

            predsb = proT.tile([1, cfg.b_pad], F32, tag="pred", name="pred")
            for q in range(math.ceil(cfg.b_pad / 512)):
                qs = slice(q * 512, min((q + 1) * 512, cfg.b_pad))
                qn = qs.stop - qs.start
                ps = prps.tile([P, 512], F32, tag="gps", name="finps",
                               space="PSUM")
                MMX(ps[:, :qn], lhsT=self.sW["Wlin2T"][:, :], rhs=outT[:, qs],
                    start=True, stop=True)
                o2 = pro.tile([P, 512], FT, tag="o2", name="o2")
                nc.vector.tensor_scalar(out=o2[:, :qn], in0=ps[:, :qn],
                                        scalar1=self.cc("b2"), scalar2=None,
                                        op0=ALU.add)
                ph = prps.tile([P, 512], F32, tag="gps", name="finph",
                               space="PSUM")
                MMX(ph[0:1, :qn], lhsT=self.sW["WheadT"][:, :], rhs=o2[:, :qn],
                    start=True, stop=True)
                nc.vector.tensor_scalar(out=predsb[:, qs], in0=ph[0:1, :qn],
                                        scalar1=float(self.b_head),
                                        scalar2=None, op0=ALU.add)
            nc.sync.dma_start(out=self.outp[:, :], in_=predsb[:, :])

    def build(self, phases=5):
        from contextlib import ExitStack
        self.declare()
        with tile.TileContext(self.nc) as tc:
            with ExitStack() as stack:
                with self.nc.named_scope("w_load"):
                    self.load_weights(tc, stack)
                if phases >= 1:
                    with self.nc.named_scope("p0_lin1"):
                        self.phase0(tc)
                if phases >= 2:
                    with self.nc.named_scope("edge1"):
                        self.edge_layer(tc, 1)
                if phases >= 3:
                    with self.nc.named_scope("exchange"):
                        if self.use_exch:
                            self.exchange(tc)
                        else:
                            self.allgather()
                if phases >= 4:
                    with self.nc.named_scope("edge2"):
                        self.edge_layer(tc, 2)
                if phases >= 5:
                    with self.nc.named_scope("readout"):
                        self.readout(tc)
                else:
                    with tc.tile_pool(name="stub", bufs=1) as pstub:
                        z = pstub.tile([1, self.cfg.b_pad], F32, tag="z",
                                       name="z")
                        self.nc.gpsimd.memset(z[:, :], 0.0)
                        self.nc.sync.dma_start(out=self.outp[:, :],
                                               in_=z[:, :])
        self.nc.finalize()
        return self.nc


def build(cfg, cvec_idx, b_head, cvech_idx=None):
    import os
    return Builder(cfg, cvec_idx, b_head, cvech_idx).build(
        phases=int(os.environ.get("K_PHASES", "5")))


def kernel(**inputs):
    cfg, in_maps, meta = preprocess(inputs, NCORES)
    nc = build(cfg, meta["cvec_idx"], meta["b_head"], meta["cvech_idx"])
    res = run_bass_kernel_spmd(nc, in_maps, core_ids=list(range(NCORES)))
    outs = []
    for c in range(NCORES):
        outs.append(res.results[c]["out"].reshape(-1)[:cfg.b_core])
    return np.concatenate(outs).astype(np.float32)

